# revision 6
# baseline (speedup 1.0000x reference)
"""Trainium2 Bass kernel for nn_AttentionEdgeDecoder.

Reference computation (per batch b):
  hn = h[b,:4096,:], hg = h[b,4096,:]
  q = hg @ W_q  (single query, 8 heads x 16 dims)
  k,v = hn @ W_kv ; attn = softmax(q.k/sqrt(16)) ; y = attn.v
  mh = y @ W_mhc ; y2[i] = <mh, hn[i]>             (4096 scalars)
  e[i,j] = y2[j]*W_lin[0,0] + y2[i]*W_lin[1,0]     (4096x4096 output)

Output is 4*4096^2*4B = 268MB -> HBM-write bound. Sharding: 8 cores =
4 batches x 2 row-halves; each core computes y2[b] redundantly (tiny) and
streams its (2048, 4096) block of e to DRAM at the per-core DMA-engine
limit (16 engines x ~27GB/s = ~430GB/s).

v3 layout, driven by perfetto/NTFF traces:
 - R = W0*y2[j] broadcast to 128 partitions lives ENTIRELY in PSUM
   (16KB/partition = all 8 banks, own pool phase) -> no PSUM->SBUF
   copies on the critical path; the e-tile adds read PSUM directly.
 - first output row-tile is added + DMA'd in 512-col pieces that chase
   the 8 R matmuls, so HBM writes start before R finishes.
 - remaining output in chunks of 1,2,4,4,4 row-tiles: partition p of a
   chunk holds TPC consecutive output rows -> one contiguous 16*TPC KB
   DMA descriptor (hrT is host-permuted so col[p] matches).
 - inputs stream in 4 chunks on both HWDGE rings; sT/exp/u are
   software-pipelined per 4-column-block group behind the DMA.
 - softmax denominator comes free from a ones-column appended to hnp.

TensorEngine formulation (out = lhsT.T @ rhs):
  q_col   = matmul(lhsT=W_q, rhs=hg_col)                  [128,1]  f32
  Qh      = headmask * q_col   (block-diag scatter)       [128,8]  f32
  Wqeff   = matmul(lhsT=WkT, rhs=Qh) = Wk @ Qh            [128,8]  ->bf16
  sT      = matmul(lhsT=hnT_chunk, rhs=Wqeff)             [4096,8] bf16 mm
  pT      = exp(0.25*sT)      (no max-subtract: |s/4| < 8)         ->bf16
  u'      = sum_chunks matmul(lhsT=pT_chunk, rhs=[hn|1])  [8,129]
  rs      = 1/u'[:,128] ; ubar = u'[:, :128] * rs -> bf16 [8,128]
  uT      = PE-transpose(ubar)  -> bf16                   [128,8]
  ymatT   = matmul(lhsT=Wv_bf, rhs=uT)                    [128,8]
  y_col   = reduce_h(ymatT * headmask) -> bf16            [128,1]
  mh_row  = matmul(lhsT=y_col, rhs=Wmhc_bf) -> bf16       [1,128]
  mh01    = matmul(lhsT=mh_row, rhs=Wl_row)               [128,2]
  mh0_rep = ones128 * mh01[:,0]  (DVE bcast)              [128,128] bf16
  col     = matmul(lhsT=hrT_tile, rhs=mh01[:,1]) = W1*y2[rows] [128,16]
  R       = matmul(lhsT=mh0_rep, rhs=hnT) in PSUM            [128,4096]
  e_tile  = tensor_scalar_add(R_psum, col[:,t]) -> DMA out
"""

from contextlib import ExitStack

import ml_dtypes
import numpy as np

import concourse.bass as bass
import concourse.mybir as mybir
from concourse import bacc, tile
from concourse.bass_utils import run_bass_kernel_spmd

BP = 4
N = 4096
HID = 128
HP1 = HID + 1           # hn chunk width incl. ones column
H = 8
D = 16
ROWS = N // 2           # 2048 rows per core
NT = ROWS // 128        # 16 row tiles per core
NJC = N // 128          # 32 node chunks
F32 = mybir.dt.float32
BF16 = mybir.dt.bfloat16

SCHED = (1, 1, 2, 4, 4, 4)   # row-tiles per output chunk
TPCMAX = max(SCHED)
NWARM = 6               # PE warm-up matmuls

# wpack column layout (all f32)
WKT0 = 0               # WkT = W_kv[:, :128].T
WV0 = HID              # Wv  = W_kv[:, 128:]
WMHC0 = 2 * HID
WQ0 = 3 * HID
ID0 = 4 * HID          # 128x128 identity
MSK0 = 5 * HID         # head mask [128, 8]
HG0 = 5 * HID + H      # hg column
WL0 = HG0 + 1          # W_lin row (partition 0)
WPACK_COLS = WL0 + 2


def build_bass():
    nc = bacc.Bacc()

    wpack_ext = nc.declare_dram_parameter("wpack", [HID, WPACK_COLS], F32, isOutput=False)
    hnT_ext = nc.declare_dram_parameter("hnT", [HID, N], BF16, isOutput=False)
    # hnp: hn pre-packed on host to [p, jc, c] = hn[jc*128+p, c], c=128 is ones
    hnp_ext = nc.declare_dram_parameter("hnp", [128, NJC * HP1], BF16, isOutput=False)
    hrT_ext = nc.declare_dram_parameter("hrT", [HID, ROWS], BF16, isOutput=False)
    out_ext = nc.declare_dram_parameter("out", [ROWS, N], F32, isOutput=True)

    with tile.TileContext(nc) as tc, ExitStack() as ctx:
        sb = ctx.enter_context(tc.tile_pool(name="sb", bufs=1))
        small = ctx.enter_context(tc.tile_pool(name="small", bufs=1))
        epool = ctx.enter_context(tc.tile_pool(name="epool", bufs=2))

        # ---- input DMAs: sync ring wpack+hnT, scalar ring hnp+hrT ----
        wpack_sb = sb.tile([HID, WPACK_COLS], F32)
        nc.sync.dma_start(wpack_sb[:], wpack_ext[:, :])
        hnT_sb = sb.tile([HID, N], BF16)
        for k in range(4):
            nc.sync.dma_start(
                hnT_sb[:, bass.ts(k, N // 4)], hnT_ext[:, bass.ts(k, N // 4)]
            )
        hn_sb = sb.tile([128, NJC, HP1], BF16)
        hn_flat = hn_sb[:].rearrange("p a b -> p (a b)")
        for k in range(4):
            nc.scalar.dma_start(
                hn_flat[:, bass.ts(k, NJC * HP1 // 4)],
                hnp_ext[:, bass.ts(k, NJC * HP1 // 4)],
            )
        hrT_sb = sb.tile([HID, ROWS], BF16)
        nc.scalar.dma_start(hrT_sb[:], hrT_ext[:, :])

        identity = wpack_sb[:, ID0:ID0 + HID]
        mask_ap = wpack_sb[:, MSK0:MSK0 + H]

        # constants / bf16 weight copies (off the critical path)
        ones128_bf = small.tile([128, HID], BF16)
        nc.vector.memset(ones128_bf[:], 1.0)
        id_bf = small.tile([HID, HID], BF16)
        nc.vector.tensor_copy(id_bf[:], identity)
        wv_bf = small.tile([HID, HID], BF16)
        nc.vector.tensor_copy(wv_bf[:], wpack_sb[:, WV0:WV0 + HID])
        wmhc_bf = small.tile([HID, HID], BF16)
        nc.vector.tensor_copy(wmhc_bf[:], wpack_sb[:, WMHC0:WMHC0 + HID])
        wl_bf = small.tile([1, 2], BF16)
        nc.vector.tensor_copy(wl_bf[:], wpack_sb[0:1, WL0:WL0 + 2])

        col_sb = small.tile([128, NT], F32)
        mh0rep_sb = small.tile([HID, HID], BF16)
        mh1_bf = small.tile([HID, 1], BF16)

        # ================= phase A: attention prologue =================
        with tc.tile_pool(name="ps_pre", bufs=1, space="PSUM") as ps_pre:
            # PE warm-up: dependency-free matmuls right after wpack lands
            for w in range(NWARM):
                warm_ps = ps_pre.tile([128, HID], F32, tag="warm", bufs=2)
                nc.tensor.matmul(warm_ps[:], identity, identity, start=True, stop=True)

            q_ps = ps_pre.tile([HID, 1], F32, tag="tmp", bufs=2, padded_shape=[128, HID])
            nc.tensor.matmul(
                q_ps[:], wpack_sb[:, WQ0:WQ0 + HID], wpack_sb[:, HG0:HG0 + 1],
                start=True, stop=True,
            )
            q_sb = small.tile([HID, 1], F32)
            nc.scalar.copy(q_sb[:], q_ps[:])

            # Qh block-diag scatter: Qh[e, h] = mask[e, h] * q[e]
            qh_sb = small.tile([HID, H], F32)
            nc.vector.tensor_scalar_mul(qh_sb[:], mask_ap, q_sb[:])

            # Wqeff = Wk @ Qh  (cast to bf16 on the PSUM->SBUF copy)
            wqeff_ps = ps_pre.tile([HID, H], F32, tag="tmp", bufs=2, padded_shape=[128, HID])
            nc.tensor.matmul(
                wqeff_ps[:], wpack_sb[:, WKT0:WKT0 + HID], qh_sb[:], start=True, stop=True
            )
            wqeff_sb = small.tile([HID, H], BF16)
            nc.scalar.copy(wqeff_sb[:], wqeff_ps[:])

            # sT / exp / u software-pipelined in groups of 4 chunks
            sT_ps = ps_pre.tile([128, NJC, H], F32, tag="sT")
            pT_sb = small.tile([128, NJC, H], BF16)
            u_ps = ps_pre.tile([H, HP1], F32, tag="u")
            NGRP = NJC // 4
            for g in range(NGRP):
                for jc in range(g * 4, g * 4 + 4):
                    nc.tensor.matmul(
                        sT_ps[:, jc, :],
                        hnT_sb[:, bass.ts(jc, 128)],
                        wqeff_sb[:],
                        start=True,
                        stop=True,
                    )
                nc.scalar.activation(
                    pT_sb[:, g * 4:(g + 1) * 4, :],
                    sT_ps[:, g * 4:(g + 1) * 4, :],
                    mybir.ActivationFunctionType.Exp,
                    scale=0.25,
                )
                if g >= 1:
                    for jc in range((g - 1) * 4, (g - 1) * 4 + 4):
                        nc.tensor.matmul(
                            u_ps[:],
                            pT_sb[:, jc, :],
                            hn_sb[:, jc, :],
                            start=(jc == 0),
                            stop=False,
                        )
            for jc in range(NJC - 4, NJC):
                nc.tensor.matmul(
                    u_ps[:], pT_sb[:, jc, :], hn_sb[:, jc, :],
                    start=False, stop=(jc == NJC - 1),
                )

            # rs = 1/ssum directly from the ones-column of u'
            rs_sb = small.tile([H, 1], F32)
            nc.vector.reciprocal(rs_sb[:], u_ps[:, HID:HP1])
            ubar_bf = small.tile([H, HID], BF16)
            nc.vector.tensor_scalar_mul(ubar_bf[:], u_ps[:, 0:HID], rs_sb[:])

            uT_ps = ps_pre.tile([HID, H], BF16, tag="tmp", bufs=2, padded_shape=[128, HID])
            nc.tensor.transpose(uT_ps[:], ubar_bf[:], id_bf[0:H, 0:H])
            uT_bf = small.tile([HID, H], BF16)
            nc.vector.tensor_copy(uT_bf[:], uT_ps[:])

            # ymatT = Wv.T @ uT  -> [e, h]
            ymatT_ps = ps_pre.tile([HID, H], F32, tag="tmp", bufs=2, padded_shape=[128, HID])
            nc.tensor.matmul(ymatT_ps[:], wv_bf[:], uT_bf[:], start=True, stop=True)
            # y_col[e] = ymatT[e, head(e)] = sum_h ymatT[e, h] * mask[e, h]
            ymm_sb = small.tile([HID, H], F32)
            y_bf = small.tile([HID, 1], BF16)
            nc.vector.tensor_mul(ymm_sb[:], ymatT_ps[:], mask_ap)
            with nc.allow_low_precision(reason="y is O(1); bf16 out is fine"):
                nc.vector.tensor_reduce(
                    y_bf[:], ymm_sb[:], axis=mybir.AxisListType.X, op=mybir.AluOpType.add
                )

            # mh_row = y.T @ W_mhc
            mh_ps = ps_pre.tile([1, HID], F32, tag="tmp", bufs=2, padded_shape=[128, HID])
            nc.tensor.matmul(mh_ps[:], y_bf[:], wmhc_bf[:], start=True, stop=True)
            mh_bf = small.tile([1, HID], BF16)
            nc.vector.tensor_copy(mh_bf[:], mh_ps[:])

            # mh01[c, :] = [W0*mh[c], W1*mh[c]]  (K=1 transpose-ish matmul)
            mh01_ps = ps_pre.tile([HID, 2], F32, tag="tmp", bufs=2, padded_shape=[128, HID])
            nc.tensor.matmul(mh01_ps[:], mh_bf[:], wl_bf[0:1, 0:2], start=True, stop=True)
            mh0_col = small.tile([HID, 1], F32)
            nc.scalar.copy(mh0_col[:], mh01_ps[:, 0:1])
            nc.vector.tensor_copy(mh1_bf[:], mh01_ps[:, 1:2])
            # mh0_rep[c, p] = W0*mh[c]  (DVE per-partition broadcast)
            nc.vector.tensor_scalar_mul(mh0rep_sb[:], ones128_bf[:], mh0_col[:])

            # col[p, t] = W1*y2[perm row] (host-permuted hrT matches SCHED)
            col_ps = ps_pre.tile([128, NT], F32, tag="col")
            for t in range(NT):
                nc.tensor.matmul(
                    col_ps[:, t:t + 1],
                    hrT_sb[:, bass.ts(t, 128)],
                    mh1_bf[:],
                    start=True,
                    stop=True,
                )
            nc.vector.tensor_copy(col_sb[:], col_ps[:])

        # ================= phase B: R in PSUM + epilogue =================
        with tc.tile_pool(name="ps_R", bufs=1, space="PSUM") as ps_R:
            r_ps = ps_R.tile([128, N], F32)
            for k in range(8):
                nc.tensor.matmul(
                    r_ps[:, bass.ts(k, 512)], mh0rep_sb[:], hnT_sb[:, bass.ts(k, 512)],
                    start=True, stop=True,
                )

            # chunk 0 (1 tile): 512-col pieces chase the R matmuls so HBM
            # writes begin before R completes
            etile0 = epool.tile([128, TPCMAX, N], F32, tag="e")
            colv0 = col_sb[:, 0:1]
            for k in range(8):
                nc.vector.tensor_scalar_add(
                    etile0[:, 0, bass.ts(k, 512)], r_ps[:, bass.ts(k, 512)], colv0
                )
                nc.sync.dma_start(
                    out_ext[0:128, bass.ts(k, 512)], etile0[:, 0, bass.ts(k, 512)]
                )

            # remaining chunks per SCHED
            r0 = 128
            cidx = 1
            for c, tpc in enumerate(SCHED[1:]):
                etile = epool.tile([128, TPCMAX, N], F32, tag="e")
                for s in range(tpc):
                    colv = col_sb[:, cidx + s:cidx + s + 1]
                    nc.vector.tensor_scalar_add(etile[:, s, :], r_ps[:], colv)
                dst = out_ext[r0:r0 + tpc * 128, :].rearrange(
                    "(p s) j -> p s j", p=128, s=tpc
                )
                nc.sync.dma_start(dst, etile[:, 0:tpc, :])
                r0 += tpc * 128
                cidx += tpc

    nc.finalize()
    return nc


_CACHED = {}


def _get_nc():
    if "nc" not in _CACHED:
        _CACHED["nc"] = build_bass()
    return _CACHED["nc"]


def _make_wpack(W_q, W_kv, W_mhc, W_lin):
    wpack = np.zeros((HID, WPACK_COLS), dtype=np.float32)
    wpack[:, WKT0:WKT0 + HID] = W_kv[:, :HID].T
    wpack[:, WV0:WV0 + HID] = W_kv[:, HID:]
    wpack[:, WMHC0:WMHC0 + HID] = W_mhc
    wpack[:, WQ0:WQ0 + HID] = W_q
    wpack[:, ID0:ID0 + HID] = np.eye(HID, dtype=np.float32)
    for hh in range(H):
        wpack[hh * D:(hh + 1) * D, MSK0 + hh] = 1.0
    wpack[0, WL0] = W_lin[0, 0]
    wpack[0, WL0 + 1] = W_lin[1, 0]
    return wpack


def _row_perm():
    # hrT column cidx*128+p  <->  local output row r0 + tpc*p + s
    perm = np.empty(ROWS, dtype=np.int64)
    cidx = 0
    r0 = 0
    for tpc in SCHED:
        for s in range(tpc):
            perm[cidx * 128:(cidx + 1) * 128] = r0 + tpc * np.arange(128) + s
            cidx += 1
        r0 += tpc * 128
    return perm


_PERM = _row_perm()


def kernel(h, W_q, W_kv, W_mhc, W_lin, _trace=False):
    h = np.ascontiguousarray(np.asarray(h, dtype=np.float32))
    W_q = np.asarray(W_q, dtype=np.float32)
    W_kv = np.asarray(W_kv, dtype=np.float32)
    W_mhc = np.asarray(W_mhc, dtype=np.float32)
    W_lin = np.asarray(W_lin, dtype=np.float32)

    nc = _get_nc()
    wpack0 = _make_wpack(W_q, W_kv, W_mhc, W_lin)

    in_maps = []
    for core in range(8):
        b, half = core // 2, core % 2
        hn = h[b, :N, :]
        wp = wpack0.copy()
        wp[:, HG0] = h[b, N, :]
        hnb = hn.astype(ml_dtypes.bfloat16)
        # hnp[p, jc*129 + c] = hn[jc*128 + p, c]; column 128 = 1.0
        hnp = np.ones((128, NJC, HP1), dtype=ml_dtypes.bfloat16)
        hnp[:, :, :HID] = hnb.reshape(NJC, 128, HID).transpose(1, 0, 2)
        hnp = np.ascontiguousarray(hnp.reshape(128, NJC * HP1))
        hr = hnb[half * ROWS:(half + 1) * ROWS, :][_PERM]
        in_maps.append(
            {
                "wpack": wp,
                "hnT": np.ascontiguousarray(hnb.T),
                "hnp": hnp,
                "hrT": np.ascontiguousarray(hr.T),
            }
        )

    import time as _time

    kw = {}
    if _trace:
        import os

        kw = {"tmpdir": "/tmp/ktrace_" + str(os.getpid())}
        os.makedirs(kw["tmpdir"], exist_ok=True)
        print("[kernel] trace dir:", kw["tmpdir"], flush=True)
    _t = _time.time()
    print("[kernel] launching run_bass_kernel_spmd", flush=True)
    res = run_bass_kernel_spmd(nc, in_maps, core_ids=list(range(8)), trace=_trace, **kw)
    print(f"[kernel] run_bass_kernel_spmd done in {_time.time()-_t:.1f}s", flush=True)

    out = np.empty((BP, N * N, 1), dtype=np.float32)
    for core in range(8):
        b, half = core // 2, core % 2
        blk = res.results[core]["out"]  # (2048, 4096)
        out[b, half * ROWS * N:(half + 1) * ROWS * N, 0] = blk.ravel()
    if _trace:
        return out, res
    return out


# revision 7
# speedup vs baseline: 1.0244x; 1.0244x over previous
"""Trainium2 Bass kernel for nn_AttentionEdgeDecoder.

Reference computation (per batch b):
  hn = h[b,:4096,:], hg = h[b,4096,:]
  q = hg @ W_q  (single query, 8 heads x 16 dims)
  k,v = hn @ W_kv ; attn = softmax(q.k/sqrt(16)) ; y = attn.v
  mh = y @ W_mhc ; y2[i] = <mh, hn[i]>             (4096 scalars)
  e[i,j] = y2[j]*W_lin[0,0] + y2[i]*W_lin[1,0]     (4096x4096 output)

Output is 4*4096^2*4B = 268MB -> HBM-write bound. Sharding: 8 cores =
4 batches x 2 row-halves; each core computes y2[b] redundantly (tiny) and
streams its (2048, 4096) block of e to DRAM at the per-core DMA-engine
limit (16 engines x ~27GB/s = ~430GB/s).

v3 layout, driven by perfetto/NTFF traces:
 - R = W0*y2[j] broadcast to 128 partitions lives ENTIRELY in PSUM
   (16KB/partition = all 8 banks, own pool phase) -> no PSUM->SBUF
   copies on the critical path; the e-tile adds read PSUM directly.
 - first output row-tile is added + DMA'd in 512-col pieces that chase
   the 8 R matmuls, so HBM writes start before R finishes.
 - remaining output in chunks of 1,2,4,4,4 row-tiles: partition p of a
   chunk holds TPC consecutive output rows -> one contiguous 16*TPC KB
   DMA descriptor (hrT is host-permuted so col[p] matches).
 - inputs stream in 4 chunks on both HWDGE rings; sT/exp/u are
   software-pipelined per 4-column-block group behind the DMA.
 - softmax denominator comes free from a ones-column appended to hnp.

TensorEngine formulation (out = lhsT.T @ rhs):
  q_col   = matmul(lhsT=W_q, rhs=hg_col)                  [128,1]  f32
  Qh      = headmask * q_col   (block-diag scatter)       [128,8]  f32
  Wqeff   = matmul(lhsT=WkT, rhs=Qh) = Wk @ Qh            [128,8]  ->bf16
  sT      = matmul(lhsT=hnT_chunk, rhs=Wqeff)             [4096,8] bf16 mm
  pT      = exp(0.25*sT)      (no max-subtract: |s/4| < 8)         ->bf16
  u'      = sum_chunks matmul(lhsT=pT_chunk, rhs=[hn|1])  [8,129]
  rs      = 1/u'[:,128] ; ubar = u'[:, :128] * rs -> bf16 [8,128]
  uT      = PE-transpose(ubar)  -> bf16                   [128,8]
  ymatT   = matmul(lhsT=Wv_bf, rhs=uT)                    [128,8]
  y_col   = reduce_h(ymatT * headmask) -> bf16            [128,1]
  mh_row  = matmul(lhsT=y_col, rhs=Wmhc_bf) -> bf16       [1,128]
  mh01    = matmul(lhsT=mh_row, rhs=Wl_row)               [128,2]
  mh0_rep = ones128 * mh01[:,0]  (DVE bcast)              [128,128] bf16
  col     = matmul(lhsT=hrT_tile, rhs=mh01[:,1]) = W1*y2[rows] [128,16]
  R       = matmul(lhsT=mh0_rep, rhs=hnT) in PSUM            [128,4096]
  e_tile  = tensor_scalar_add(R_psum, col[:,t]) -> DMA out
"""

from contextlib import ExitStack

import ml_dtypes
import numpy as np

import concourse.bass as bass
import concourse.mybir as mybir
from concourse import bacc, tile
from concourse.bass_utils import run_bass_kernel_spmd

BP = 4
N = 4096
HID = 128
HP1 = HID + 1           # hn chunk width incl. ones column
H = 8
D = 16
ROWS = N // 2           # 2048 rows per core
NT = ROWS // 128        # 16 row tiles per core
NJC = N // 128          # 32 node chunks
F32 = mybir.dt.float32
BF16 = mybir.dt.bfloat16

SCHED = (1, 1, 2, 4, 4, 4)   # row-tiles per output chunk
TPCMAX = max(SCHED)
NWARM = 6               # PE warm-up matmuls

# wpack column layout (all f32)
WKT0 = 0               # WkT = W_kv[:, :128].T
WV0 = HID              # Wv  = W_kv[:, 128:]
WMHC0 = 2 * HID
WQ0 = 3 * HID
ID0 = 4 * HID          # 128x128 identity
MSK0 = 5 * HID         # head mask [128, 8]
HG0 = 5 * HID + H      # hg column
WL0 = HG0 + 1          # W_lin row (partition 0)
WPACK_COLS = WL0 + 2


def build_bass():
    nc = bacc.Bacc()

    wpack_ext = nc.declare_dram_parameter("wpack", [HID, WPACK_COLS], F32, isOutput=False)
    hnT_ext = nc.declare_dram_parameter("hnT", [HID, N], BF16, isOutput=False)
    # hnp: hn pre-packed on host to [p, jc, c] = hn[jc*128+p, c], c=128 is ones
    hnp_ext = nc.declare_dram_parameter("hnp", [128, NJC * HP1], BF16, isOutput=False)
    hrT_ext = nc.declare_dram_parameter("hrT", [HID, ROWS], BF16, isOutput=False)
    out_ext = nc.declare_dram_parameter("out", [ROWS, N], F32, isOutput=True)

    with tile.TileContext(nc) as tc, ExitStack() as ctx:
        sb = ctx.enter_context(tc.tile_pool(name="sb", bufs=1))
        small = ctx.enter_context(tc.tile_pool(name="small", bufs=1))
        epool = ctx.enter_context(tc.tile_pool(name="epool", bufs=2))

        # ---- input DMAs: sync ring wpack+hnT, scalar ring hnp+hrT ----
        wpack_sb = sb.tile([HID, WPACK_COLS], F32)
        nc.sync.dma_start(wpack_sb[:], wpack_ext[:, :])
        hnT_sb = sb.tile([HID, N], BF16)
        for k in range(4):
            nc.sync.dma_start(
                hnT_sb[:, bass.ts(k, N // 4)], hnT_ext[:, bass.ts(k, N // 4)]
            )
        hn_sb = sb.tile([128, NJC, HP1], BF16)
        hn_flat = hn_sb[:].rearrange("p a b -> p (a b)")
        for k in range(4):
            nc.scalar.dma_start(
                hn_flat[:, bass.ts(k, NJC * HP1 // 4)],
                hnp_ext[:, bass.ts(k, NJC * HP1 // 4)],
            )
        hrT_sb = sb.tile([HID, ROWS], BF16)
        nc.scalar.dma_start(hrT_sb[:], hrT_ext[:, :])

        identity = wpack_sb[:, ID0:ID0 + HID]
        mask_ap = wpack_sb[:, MSK0:MSK0 + H]

        # constants / bf16 weight copies (off the critical path)
        ones128_bf = small.tile([128, HID], BF16)
        nc.vector.memset(ones128_bf[:], 1.0)
        id_bf = small.tile([HID, HID], BF16)
        nc.vector.tensor_copy(id_bf[:], identity)
        wv_bf = small.tile([HID, HID], BF16)
        nc.vector.tensor_copy(wv_bf[:], wpack_sb[:, WV0:WV0 + HID])
        wmhc_bf = small.tile([HID, HID], BF16)
        nc.vector.tensor_copy(wmhc_bf[:], wpack_sb[:, WMHC0:WMHC0 + HID])
        wl_bf = small.tile([1, 2], BF16)
        nc.vector.tensor_copy(wl_bf[:], wpack_sb[0:1, WL0:WL0 + 2])

        col_sb = small.tile([128, NT], F32)
        mh0rep_sb = small.tile([HID, HID], BF16)
        mh1_bf = small.tile([HID, 1], BF16)

        # ================= phase A: attention prologue =================
        with tc.tile_pool(name="ps_pre", bufs=1, space="PSUM") as ps_pre:
            # PE warm-up: dependency-free matmuls right after wpack lands
            for w in range(NWARM):
                warm_ps = ps_pre.tile([128, HID], F32, tag="warm", bufs=2)
                nc.tensor.matmul(warm_ps[:], identity, identity, start=True, stop=True)

            q_ps = ps_pre.tile([HID, 1], F32, tag="tmp", bufs=2, padded_shape=[128, HID])
            nc.tensor.matmul(
                q_ps[:], wpack_sb[:, WQ0:WQ0 + HID], wpack_sb[:, HG0:HG0 + 1],
                start=True, stop=True,
            )
            q_sb = small.tile([HID, 1], F32)
            nc.scalar.copy(q_sb[:], q_ps[:])

            # Qh block-diag scatter: Qh[e, h] = mask[e, h] * q[e]
            qh_sb = small.tile([HID, H], F32)
            nc.vector.tensor_scalar_mul(qh_sb[:], mask_ap, q_sb[:])

            # Wqeff = Wk @ Qh  (cast to bf16 on the PSUM->SBUF copy)
            wqeff_ps = ps_pre.tile([HID, H], F32, tag="tmp", bufs=2, padded_shape=[128, HID])
            nc.tensor.matmul(
                wqeff_ps[:], wpack_sb[:, WKT0:WKT0 + HID], qh_sb[:], start=True, stop=True
            )
            wqeff_sb = small.tile([HID, H], BF16)
            nc.scalar.copy(wqeff_sb[:], wqeff_ps[:])

            # sT / exp / u software-pipelined in groups of 4 chunks
            sT_ps = ps_pre.tile([128, NJC, H], F32, tag="sT")
            pT_sb = small.tile([128, NJC, H], BF16)
            u_ps = ps_pre.tile([H, HP1], F32, tag="u")
            NGRP = NJC // 4
            for g in range(NGRP):
                for jc in range(g * 4, g * 4 + 4):
                    nc.tensor.matmul(
                        sT_ps[:, jc, :],
                        hnT_sb[:, bass.ts(jc, 128)],
                        wqeff_sb[:],
                        start=True,
                        stop=True,
                    )
                nc.scalar.activation(
                    pT_sb[:, g * 4:(g + 1) * 4, :],
                    sT_ps[:, g * 4:(g + 1) * 4, :],
                    mybir.ActivationFunctionType.Exp,
                    scale=0.25,
                )
                if g >= 1:
                    for jc in range((g - 1) * 4, (g - 1) * 4 + 4):
                        nc.tensor.matmul(
                            u_ps[:],
                            pT_sb[:, jc, :],
                            hn_sb[:, jc, :],
                            start=(jc == 0),
                            stop=False,
                        )
            for jc in range(NJC - 4, NJC):
                nc.tensor.matmul(
                    u_ps[:], pT_sb[:, jc, :], hn_sb[:, jc, :],
                    start=False, stop=(jc == NJC - 1),
                )

            # rs = 1/ssum directly from the ones-column of u'
            rs_sb = small.tile([H, 1], F32)
            nc.vector.reciprocal(rs_sb[:], u_ps[:, HID:HP1])
            ubar_bf = small.tile([H, HID], BF16)
            nc.vector.tensor_scalar_mul(ubar_bf[:], u_ps[:, 0:HID], rs_sb[:])

            uT_ps = ps_pre.tile([HID, H], BF16, tag="tmp", bufs=2, padded_shape=[128, HID])
            nc.tensor.transpose(uT_ps[:], ubar_bf[:], id_bf[0:H, 0:H])
            uT_bf = small.tile([HID, H], BF16)
            nc.vector.tensor_copy(uT_bf[:], uT_ps[:])

            # ymatT = Wv.T @ uT  -> [e, h]
            ymatT_ps = ps_pre.tile([HID, H], F32, tag="tmp", bufs=2, padded_shape=[128, HID])
            nc.tensor.matmul(ymatT_ps[:], wv_bf[:], uT_bf[:], start=True, stop=True)
            # y_col[e] = ymatT[e, head(e)] = sum_h ymatT[e, h] * mask[e, h]
            ymm_sb = small.tile([HID, H], F32)
            y_bf = small.tile([HID, 1], BF16)
            nc.vector.tensor_mul(ymm_sb[:], ymatT_ps[:], mask_ap)
            with nc.allow_low_precision(reason="y is O(1); bf16 out is fine"):
                nc.vector.tensor_reduce(
                    y_bf[:], ymm_sb[:], axis=mybir.AxisListType.X, op=mybir.AluOpType.add
                )

            # mh_row = y.T @ W_mhc
            mh_ps = ps_pre.tile([1, HID], F32, tag="tmp", bufs=2, padded_shape=[128, HID])
            nc.tensor.matmul(mh_ps[:], y_bf[:], wmhc_bf[:], start=True, stop=True)
            mh_bf = small.tile([1, HID], BF16)
            nc.vector.tensor_copy(mh_bf[:], mh_ps[:])

            # mh01[c, :] = [W0*mh[c], W1*mh[c]]  (K=1 transpose-ish matmul)
            mh01_ps = ps_pre.tile([HID, 2], F32, tag="tmp", bufs=2, padded_shape=[128, HID])
            nc.tensor.matmul(mh01_ps[:], mh_bf[:], wl_bf[0:1, 0:2], start=True, stop=True)
            mh0_col = small.tile([HID, 1], F32)
            nc.scalar.copy(mh0_col[:], mh01_ps[:, 0:1])
            nc.vector.tensor_copy(mh1_bf[:], mh01_ps[:, 1:2])
            # mh0_rep[c, p] = W0*mh[c]  (DVE per-partition broadcast)
            nc.vector.tensor_scalar_mul(mh0rep_sb[:], ones128_bf[:], mh0_col[:])

            # col[p, t] = W1*y2[perm row] (host-permuted hrT matches SCHED)
            col_ps = ps_pre.tile([128, NT], F32, tag="col")
            for t in range(NT):
                nc.tensor.matmul(
                    col_ps[:, t:t + 1],
                    hrT_sb[:, bass.ts(t, 128)],
                    mh1_bf[:],
                    start=True,
                    stop=True,
                )
            nc.vector.tensor_copy(col_sb[:], col_ps[:])

        # ================= phase B: R in PSUM + epilogue =================
        # DVE reads PSUM at ~half its SBUF rate, so only the first two
        # row-tiles add directly from PSUM (512-col pieces chasing the R
        # matmuls -> HBM writes start before R completes). Meanwhile the
        # scalar engine copies R into SBUF; all later tiles add from SBUF
        # at full DVE rate to stay ahead of the DMA drain.
        r_sb = sb.tile([128, N], F32)
        with tc.tile_pool(name="ps_R", bufs=1, space="PSUM") as ps_R:
            r_ps = ps_R.tile([128, N], F32)
            for k in range(8):
                nc.tensor.matmul(
                    r_ps[:, bass.ts(k, 512)], mh0rep_sb[:], hnT_sb[:, bass.ts(k, 512)],
                    start=True, stop=True,
                )

            etile0 = epool.tile([128, TPCMAX, N], F32, tag="e")
            etile1 = epool.tile([128, TPCMAX, N], F32, tag="e")
            for k in range(8):
                nc.vector.tensor_scalar_add(
                    etile0[:, 0, bass.ts(k, 512)], r_ps[:, bass.ts(k, 512)],
                    col_sb[:, 0:1],
                )
                nc.sync.dma_start(
                    out_ext[0:128, bass.ts(k, 512)], etile0[:, 0, bass.ts(k, 512)]
                )
                nc.scalar.copy(r_sb[:, bass.ts(k, 512)], r_ps[:, bass.ts(k, 512)])
            for k in range(8):
                nc.vector.tensor_scalar_add(
                    etile1[:, 0, bass.ts(k, 512)], r_ps[:, bass.ts(k, 512)],
                    col_sb[:, 1:2],
                )
                nc.sync.dma_start(
                    out_ext[128:256, bass.ts(k, 512)], etile1[:, 0, bass.ts(k, 512)]
                )

        # remaining chunks per SCHED, added from SBUF
        r0 = 256
        cidx = 2
        for tpc in SCHED[2:]:
            etile = epool.tile([128, TPCMAX, N], F32, tag="e")
            for s in range(tpc):
                colv = col_sb[:, cidx + s:cidx + s + 1]
                nc.vector.tensor_scalar_add(etile[:, s, :], r_sb[:], colv)
            dst = out_ext[r0:r0 + tpc * 128, :].rearrange(
                "(p s) j -> p s j", p=128, s=tpc
            )
            nc.sync.dma_start(dst, etile[:, 0:tpc, :])
            r0 += tpc * 128
            cidx += tpc

    nc.finalize()
    return nc


_CACHED = {}


def _get_nc():
    if "nc" not in _CACHED:
        _CACHED["nc"] = build_bass()
    return _CACHED["nc"]


def _make_wpack(W_q, W_kv, W_mhc, W_lin):
    wpack = np.zeros((HID, WPACK_COLS), dtype=np.float32)
    wpack[:, WKT0:WKT0 + HID] = W_kv[:, :HID].T
    wpack[:, WV0:WV0 + HID] = W_kv[:, HID:]
    wpack[:, WMHC0:WMHC0 + HID] = W_mhc
    wpack[:, WQ0:WQ0 + HID] = W_q
    wpack[:, ID0:ID0 + HID] = np.eye(HID, dtype=np.float32)
    for hh in range(H):
        wpack[hh * D:(hh + 1) * D, MSK0 + hh] = 1.0
    wpack[0, WL0] = W_lin[0, 0]
    wpack[0, WL0 + 1] = W_lin[1, 0]
    return wpack


def _row_perm():
    # hrT column cidx*128+p  <->  local output row r0 + tpc*p + s
    perm = np.empty(ROWS, dtype=np.int64)
    cidx = 0
    r0 = 0
    for tpc in SCHED:
        for s in range(tpc):
            perm[cidx * 128:(cidx + 1) * 128] = r0 + tpc * np.arange(128) + s
            cidx += 1
        r0 += tpc * 128
    return perm


_PERM = _row_perm()


def kernel(h, W_q, W_kv, W_mhc, W_lin, _trace=False):
    h = np.ascontiguousarray(np.asarray(h, dtype=np.float32))
    W_q = np.asarray(W_q, dtype=np.float32)
    W_kv = np.asarray(W_kv, dtype=np.float32)
    W_mhc = np.asarray(W_mhc, dtype=np.float32)
    W_lin = np.asarray(W_lin, dtype=np.float32)

    nc = _get_nc()
    wpack0 = _make_wpack(W_q, W_kv, W_mhc, W_lin)

    in_maps = []
    for core in range(8):
        b, half = core // 2, core % 2
        hn = h[b, :N, :]
        wp = wpack0.copy()
        wp[:, HG0] = h[b, N, :]
        hnb = hn.astype(ml_dtypes.bfloat16)
        # hnp[p, jc*129 + c] = hn[jc*128 + p, c]; column 128 = 1.0
        hnp = np.ones((128, NJC, HP1), dtype=ml_dtypes.bfloat16)
        hnp[:, :, :HID] = hnb.reshape(NJC, 128, HID).transpose(1, 0, 2)
        hnp = np.ascontiguousarray(hnp.reshape(128, NJC * HP1))
        hr = hnb[half * ROWS:(half + 1) * ROWS, :][_PERM]
        in_maps.append(
            {
                "wpack": wp,
                "hnT": np.ascontiguousarray(hnb.T),
                "hnp": hnp,
                "hrT": np.ascontiguousarray(hr.T),
            }
        )

    import time as _time

    kw = {}
    if _trace:
        import os

        kw = {"tmpdir": "/tmp/ktrace_" + str(os.getpid())}
        os.makedirs(kw["tmpdir"], exist_ok=True)
        print("[kernel] trace dir:", kw["tmpdir"], flush=True)
    _t = _time.time()
    print("[kernel] launching run_bass_kernel_spmd", flush=True)
    res = run_bass_kernel_spmd(nc, in_maps, core_ids=list(range(8)), trace=_trace, **kw)
    print(f"[kernel] run_bass_kernel_spmd done in {_time.time()-_t:.1f}s", flush=True)

    out = np.empty((BP, N * N, 1), dtype=np.float32)
    for core in range(8):
        b, half = core // 2, core % 2
        blk = res.results[core]["out"]  # (2048, 4096)
        out[b, half * ROWS * N:(half + 1) * ROWS * N, 0] = blk.ravel()
    if _trace:
        return out, res
    return out


# revision 8
# speedup vs baseline: 1.1256x; 1.0987x over previous
"""Trainium2 Bass kernel for nn_AttentionEdgeDecoder.

Reference computation (per batch b):
  hn = h[b,:4096,:], hg = h[b,4096,:]
  q = hg @ W_q  (single query, 8 heads x 16 dims)
  k,v = hn @ W_kv ; attn = softmax(q.k/sqrt(16)) ; y = attn.v
  mh = y @ W_mhc ; y2[i] = <mh, hn[i]>             (4096 scalars)
  e[i,j] = y2[j]*W_lin[0,0] + y2[i]*W_lin[1,0]     (4096x4096 output)

Output is 4*4096^2*4B = 268MB -> HBM-write bound. Sharding: 8 cores =
4 batches x 2 row-halves; each core computes y2[b] redundantly (tiny) and
streams its (2048, 4096) block of e to DRAM at the per-core DMA-engine
limit (16 engines x ~27GB/s = ~430GB/s).

v3 layout, driven by perfetto/NTFF traces:
 - R = W0*y2[j] broadcast to 128 partitions lives ENTIRELY in PSUM
   (16KB/partition = all 8 banks, own pool phase) -> no PSUM->SBUF
   copies on the critical path; the e-tile adds read PSUM directly.
 - first output row-tile is added + DMA'd in 512-col pieces that chase
   the 8 R matmuls, so HBM writes start before R finishes.
 - remaining output in chunks of 1,2,4,4,4 row-tiles: partition p of a
   chunk holds TPC consecutive output rows -> one contiguous 16*TPC KB
   DMA descriptor (hrT is host-permuted so col[p] matches).
 - inputs stream in 4 chunks on both HWDGE rings; sT/exp/u are
   software-pipelined per 4-column-block group behind the DMA.
 - softmax denominator comes free from a ones-column appended to hnp.

TensorEngine formulation (out = lhsT.T @ rhs):
  q_col   = matmul(lhsT=W_q, rhs=hg_col)                  [128,1]  f32
  Qh      = headmask * q_col   (block-diag scatter)       [128,8]  f32
  Wqeff   = matmul(lhsT=WkT, rhs=Qh) = Wk @ Qh            [128,8]  ->bf16
  sT      = matmul(lhsT=hnT_chunk, rhs=Wqeff)             [4096,8] bf16 mm
  pT      = exp(0.25*sT)      (no max-subtract: |s/4| < 8)         ->bf16
  u'      = sum_chunks matmul(lhsT=pT_chunk, rhs=[hn|1])  [8,129]
  rs      = 1/u'[:,128] ; ubar = u'[:, :128] * rs -> bf16 [8,128]
  uT      = PE-transpose(ubar)  -> bf16                   [128,8]
  ymatT   = matmul(lhsT=Wv_bf, rhs=uT)                    [128,8]
  y_col   = reduce_h(ymatT * headmask) -> bf16            [128,1]
  mh_row  = matmul(lhsT=y_col, rhs=Wmhc_bf) -> bf16       [1,128]
  mh01    = matmul(lhsT=mh_row, rhs=Wl_row)               [128,2]
  mh0_rep = ones128 * mh01[:,0]  (DVE bcast)              [128,128] bf16
  col     = matmul(lhsT=hrT_tile, rhs=mh01[:,1]) = W1*y2[rows] [128,16]
  R       = matmul(lhsT=mh0_rep, rhs=hnT) in PSUM            [128,4096]
  e_tile  = tensor_scalar_add(R_psum, col[:,t]) -> DMA out
"""

from contextlib import ExitStack

import ml_dtypes
import numpy as np

import concourse.bass as bass
import concourse.mybir as mybir
from concourse import bacc, tile
from concourse.bass_utils import run_bass_kernel_spmd

BP = 4
N = 4096
HID = 128
HP1 = HID + 1           # hn chunk width incl. ones column
H = 8
D = 16
ROWS = N // 2           # 2048 rows per core
NT = ROWS // 128        # 16 row tiles per core
NJC = N // 128          # 32 node chunks
F32 = mybir.dt.float32
BF16 = mybir.dt.bfloat16

SCHED = (1, 1, 2, 4, 4, 4)   # row-tiles per output chunk
TPCMAX = max(SCHED)
NWARM = 6               # PE warm-up matmuls

# wpack column layout (all f32)
WKT0 = 0               # WkT = W_kv[:, :128].T
WV0 = HID              # Wv  = W_kv[:, 128:]
WMHC0 = 2 * HID
WQ0 = 3 * HID
ID0 = 4 * HID          # 128x128 identity
MSK0 = 5 * HID         # head mask [128, 8]
HG0 = 5 * HID + H      # hg column
WL0 = HG0 + 1          # W_lin row (partition 0)
WPACK_COLS = WL0 + 2


def build_bass():
    nc = bacc.Bacc()

    wpack_ext = nc.declare_dram_parameter("wpack", [HID, WPACK_COLS], F32, isOutput=False)
    hnT_ext = nc.declare_dram_parameter("hnT", [HID, N], BF16, isOutput=False)
    # hnp: hn pre-packed on host to [p, jc, c] = hn[jc*128+p, c], c=128 is ones
    hnp_ext = nc.declare_dram_parameter("hnp", [128, NJC * HP1], BF16, isOutput=False)
    hrT_ext = nc.declare_dram_parameter("hrT", [HID, ROWS], BF16, isOutput=False)
    out_ext = nc.declare_dram_parameter("out", [ROWS, N], F32, isOutput=True)

    with tile.TileContext(nc) as tc, ExitStack() as ctx:
        sb = ctx.enter_context(tc.tile_pool(name="sb", bufs=1))
        small = ctx.enter_context(tc.tile_pool(name="small", bufs=1))
        epool = ctx.enter_context(tc.tile_pool(name="epool", bufs=2))

        # ---- input DMAs: sync ring wpack+hnT, scalar ring hnp+hrT ----
        wpack_sb = sb.tile([HID, WPACK_COLS], F32)
        nc.sync.dma_start(wpack_sb[:], wpack_ext[:, :])
        hnT_sb = sb.tile([HID, N], BF16)
        for k in range(4):
            nc.sync.dma_start(
                hnT_sb[:, bass.ts(k, N // 4)], hnT_ext[:, bass.ts(k, N // 4)]
            )
        hn_sb = sb.tile([128, NJC, HP1], BF16)
        hn_flat = hn_sb[:].rearrange("p a b -> p (a b)")
        for k in range(4):
            nc.scalar.dma_start(
                hn_flat[:, bass.ts(k, NJC * HP1 // 4)],
                hnp_ext[:, bass.ts(k, NJC * HP1 // 4)],
            )
        hrT_sb = sb.tile([HID, ROWS], BF16)
        nc.scalar.dma_start(hrT_sb[:], hrT_ext[:, :])

        identity = wpack_sb[:, ID0:ID0 + HID]
        mask_ap = wpack_sb[:, MSK0:MSK0 + H]

        # constants / bf16 weight copies (off the critical path)
        ones128_bf = small.tile([128, HID], BF16)
        nc.vector.memset(ones128_bf[:], 1.0)
        id_bf = small.tile([HID, HID], BF16)
        nc.vector.tensor_copy(id_bf[:], identity)
        wv_bf = small.tile([HID, HID], BF16)
        nc.vector.tensor_copy(wv_bf[:], wpack_sb[:, WV0:WV0 + HID])
        wmhc_bf = small.tile([HID, HID], BF16)
        nc.vector.tensor_copy(wmhc_bf[:], wpack_sb[:, WMHC0:WMHC0 + HID])
        wl_bf = small.tile([1, 2], BF16)
        nc.vector.tensor_copy(wl_bf[:], wpack_sb[0:1, WL0:WL0 + 2])

        col_sb = small.tile([128, NT], F32)
        mh0rep_sb = small.tile([HID, HID], BF16)
        mh1_bf = small.tile([HID, 1], BF16)

        # ================= phase A: attention prologue =================
        with tc.tile_pool(name="ps_pre", bufs=1, space="PSUM") as ps_pre:
            # PE warm-up: dependency-free matmuls right after wpack lands
            for w in range(NWARM):
                warm_ps = ps_pre.tile([128, HID], F32, tag="warm", bufs=2)
                nc.tensor.matmul(warm_ps[:], identity, identity, start=True, stop=True)

            q_ps = ps_pre.tile([HID, 1], F32, tag="tmp", bufs=2, padded_shape=[128, HID])
            nc.tensor.matmul(
                q_ps[:], wpack_sb[:, WQ0:WQ0 + HID], wpack_sb[:, HG0:HG0 + 1],
                start=True, stop=True,
            )
            q_sb = small.tile([HID, 1], F32)
            nc.scalar.copy(q_sb[:], q_ps[:])

            # Qh block-diag scatter: Qh[e, h] = mask[e, h] * q[e]
            qh_sb = small.tile([HID, H], F32)
            nc.vector.tensor_scalar_mul(qh_sb[:], mask_ap, q_sb[:])

            # Wqeff = Wk @ Qh  (cast to bf16 on the PSUM->SBUF copy)
            wqeff_ps = ps_pre.tile([HID, H], F32, tag="tmp", bufs=2, padded_shape=[128, HID])
            nc.tensor.matmul(
                wqeff_ps[:], wpack_sb[:, WKT0:WKT0 + HID], qh_sb[:], start=True, stop=True
            )
            wqeff_sb = small.tile([HID, H], BF16)
            nc.scalar.copy(wqeff_sb[:], wqeff_ps[:])

            # sT / exp / u software-pipelined in groups of 4 chunks
            sT_ps = ps_pre.tile([128, NJC, H], F32, tag="sT")
            pT_sb = small.tile([128, NJC, H], BF16)
            u_ps = ps_pre.tile([H, HP1], F32, tag="u")
            NGRP = NJC // 4
            for g in range(NGRP):
                for jc in range(g * 4, g * 4 + 4):
                    nc.tensor.matmul(
                        sT_ps[:, jc, :],
                        hnT_sb[:, bass.ts(jc, 128)],
                        wqeff_sb[:],
                        start=True,
                        stop=True,
                    )
                nc.scalar.activation(
                    pT_sb[:, g * 4:(g + 1) * 4, :],
                    sT_ps[:, g * 4:(g + 1) * 4, :],
                    mybir.ActivationFunctionType.Exp,
                    scale=0.25,
                )
                if g >= 1:
                    for jc in range((g - 1) * 4, (g - 1) * 4 + 4):
                        nc.tensor.matmul(
                            u_ps[:],
                            pT_sb[:, jc, :],
                            hn_sb[:, jc, :],
                            start=(jc == 0),
                            stop=False,
                        )
            for jc in range(NJC - 4, NJC):
                nc.tensor.matmul(
                    u_ps[:], pT_sb[:, jc, :], hn_sb[:, jc, :],
                    start=False, stop=(jc == NJC - 1),
                )

            # rs = 1/ssum directly from the ones-column of u'
            rs_sb = small.tile([H, 1], F32)
            nc.vector.reciprocal(rs_sb[:], u_ps[:, HID:HP1])
            ubar_bf = small.tile([H, HID], BF16)
            nc.vector.tensor_scalar_mul(ubar_bf[:], u_ps[:, 0:HID], rs_sb[:])

            uT_ps = ps_pre.tile([HID, H], BF16, tag="tmp", bufs=2, padded_shape=[128, HID])
            nc.tensor.transpose(uT_ps[:], ubar_bf[:], id_bf[0:H, 0:H])
            uT_bf = small.tile([HID, H], BF16)
            nc.vector.tensor_copy(uT_bf[:], uT_ps[:])

            # ymatT = Wv.T @ uT  -> [e, h]
            ymatT_ps = ps_pre.tile([HID, H], F32, tag="tmp", bufs=2, padded_shape=[128, HID])
            nc.tensor.matmul(ymatT_ps[:], wv_bf[:], uT_bf[:], start=True, stop=True)
            # y_col[e] = ymatT[e, head(e)] = sum_h ymatT[e, h] * mask[e, h]
            ymm_sb = small.tile([HID, H], F32)
            y_bf = small.tile([HID, 1], BF16)
            nc.vector.tensor_mul(ymm_sb[:], ymatT_ps[:], mask_ap)
            with nc.allow_low_precision(reason="y is O(1); bf16 out is fine"):
                nc.vector.tensor_reduce(
                    y_bf[:], ymm_sb[:], axis=mybir.AxisListType.X, op=mybir.AluOpType.add
                )

            # mh_row = y.T @ W_mhc
            mh_ps = ps_pre.tile([1, HID], F32, tag="tmp", bufs=2, padded_shape=[128, HID])
            nc.tensor.matmul(mh_ps[:], y_bf[:], wmhc_bf[:], start=True, stop=True)
            mh_bf = small.tile([1, HID], BF16)
            nc.vector.tensor_copy(mh_bf[:], mh_ps[:])

            # mh01[c, :] = [W0*mh[c], W1*mh[c]]  (K=1 transpose-ish matmul)
            mh01_ps = ps_pre.tile([HID, 2], F32, tag="tmp", bufs=2, padded_shape=[128, HID])
            nc.tensor.matmul(mh01_ps[:], mh_bf[:], wl_bf[0:1, 0:2], start=True, stop=True)
            mh0_col = small.tile([HID, 1], F32)
            nc.scalar.copy(mh0_col[:], mh01_ps[:, 0:1])
            nc.vector.tensor_copy(mh1_bf[:], mh01_ps[:, 1:2])
            # mh0_rep[c, p] = W0*mh[c]  (DVE per-partition broadcast)
            nc.vector.tensor_scalar_mul(mh0rep_sb[:], ones128_bf[:], mh0_col[:])

            # col[p, t] = W1*y2[perm row] (host-permuted hrT matches SCHED)
            col_ps = ps_pre.tile([128, NT], F32, tag="col")
            for t in range(NT):
                nc.tensor.matmul(
                    col_ps[:, t:t + 1],
                    hrT_sb[:, bass.ts(t, 128)],
                    mh1_bf[:],
                    start=True,
                    stop=True,
                )
            nc.vector.tensor_copy(col_sb[:], col_ps[:])

        # ================= phase B: R in PSUM + epilogue =================
        # DVE reads PSUM at ~half its SBUF rate, so only the first two
        # row-tiles add directly from PSUM (512-col pieces chasing the R
        # matmuls -> HBM writes start before R completes). Meanwhile the
        # scalar engine copies R into SBUF; all later tiles add from SBUF
        # at full DVE rate to stay ahead of the DMA drain.
        r_sb = sb.tile([128, N], F32)
        with tc.tile_pool(name="ps_R", bufs=1, space="PSUM") as ps_R:
            r_ps = ps_R.tile([128, N], F32)
            for k in range(8):
                nc.tensor.matmul(
                    r_ps[:, bass.ts(k, 512)], mh0rep_sb[:], hnT_sb[:, bass.ts(k, 512)],
                    start=True, stop=True,
                )

            # chunk 0: 512-col adds chase R in PSUM, then ONE full-width DMA
            # (16KB descriptors stream at full engine rate; 2KB pieces don't)
            etile0 = epool.tile([128, TPCMAX, N], F32, tag="e")
            for k in range(8):
                nc.vector.tensor_scalar_add(
                    etile0[:, 0, bass.ts(k, 512)], r_ps[:, bass.ts(k, 512)],
                    col_sb[:, 0:1],
                )
                nc.scalar.copy(r_sb[:, bass.ts(k, 512)], r_ps[:, bass.ts(k, 512)])
            nc.sync.dma_start(out_ext[0:128, :], etile0[:, 0, :])

        # remaining chunks per SCHED, added from SBUF
        r0 = 128
        cidx = 1
        for tpc in SCHED[1:]:
            etile = epool.tile([128, TPCMAX, N], F32, tag="e")
            for s in range(tpc):
                colv = col_sb[:, cidx + s:cidx + s + 1]
                nc.vector.tensor_scalar_add(etile[:, s, :], r_sb[:], colv)
            dst = out_ext[r0:r0 + tpc * 128, :].rearrange(
                "(p s) j -> p s j", p=128, s=tpc
            )
            nc.sync.dma_start(dst, etile[:, 0:tpc, :])
            r0 += tpc * 128
            cidx += tpc

    nc.finalize()
    return nc


_CACHED = {}


def _get_nc():
    if "nc" not in _CACHED:
        _CACHED["nc"] = build_bass()
    return _CACHED["nc"]


def _make_wpack(W_q, W_kv, W_mhc, W_lin):
    wpack = np.zeros((HID, WPACK_COLS), dtype=np.float32)
    wpack[:, WKT0:WKT0 + HID] = W_kv[:, :HID].T
    wpack[:, WV0:WV0 + HID] = W_kv[:, HID:]
    wpack[:, WMHC0:WMHC0 + HID] = W_mhc
    wpack[:, WQ0:WQ0 + HID] = W_q
    wpack[:, ID0:ID0 + HID] = np.eye(HID, dtype=np.float32)
    for hh in range(H):
        wpack[hh * D:(hh + 1) * D, MSK0 + hh] = 1.0
    wpack[0, WL0] = W_lin[0, 0]
    wpack[0, WL0 + 1] = W_lin[1, 0]
    return wpack


def _row_perm():
    # hrT column cidx*128+p  <->  local output row r0 + tpc*p + s
    perm = np.empty(ROWS, dtype=np.int64)
    cidx = 0
    r0 = 0
    for tpc in SCHED:
        for s in range(tpc):
            perm[cidx * 128:(cidx + 1) * 128] = r0 + tpc * np.arange(128) + s
            cidx += 1
        r0 += tpc * 128
    return perm


_PERM = _row_perm()


def kernel(h, W_q, W_kv, W_mhc, W_lin, _trace=False):
    h = np.ascontiguousarray(np.asarray(h, dtype=np.float32))
    W_q = np.asarray(W_q, dtype=np.float32)
    W_kv = np.asarray(W_kv, dtype=np.float32)
    W_mhc = np.asarray(W_mhc, dtype=np.float32)
    W_lin = np.asarray(W_lin, dtype=np.float32)

    nc = _get_nc()
    wpack0 = _make_wpack(W_q, W_kv, W_mhc, W_lin)

    in_maps = []
    for core in range(8):
        b, half = core // 2, core % 2
        hn = h[b, :N, :]
        wp = wpack0.copy()
        wp[:, HG0] = h[b, N, :]
        hnb = hn.astype(ml_dtypes.bfloat16)
        # hnp[p, jc*129 + c] = hn[jc*128 + p, c]; column 128 = 1.0
        hnp = np.ones((128, NJC, HP1), dtype=ml_dtypes.bfloat16)
        hnp[:, :, :HID] = hnb.reshape(NJC, 128, HID).transpose(1, 0, 2)
        hnp = np.ascontiguousarray(hnp.reshape(128, NJC * HP1))
        hr = hnb[half * ROWS:(half + 1) * ROWS, :][_PERM]
        in_maps.append(
            {
                "wpack": wp,
                "hnT": np.ascontiguousarray(hnb.T),
                "hnp": hnp,
                "hrT": np.ascontiguousarray(hr.T),
            }
        )

    import time as _time

    kw = {}
    if _trace:
        import os

        kw = {"tmpdir": "/tmp/ktrace_" + str(os.getpid())}
        os.makedirs(kw["tmpdir"], exist_ok=True)
        print("[kernel] trace dir:", kw["tmpdir"], flush=True)
    _t = _time.time()
    print("[kernel] launching run_bass_kernel_spmd", flush=True)
    res = run_bass_kernel_spmd(nc, in_maps, core_ids=list(range(8)), trace=_trace, **kw)
    print(f"[kernel] run_bass_kernel_spmd done in {_time.time()-_t:.1f}s", flush=True)

    out = np.empty((BP, N * N, 1), dtype=np.float32)
    for core in range(8):
        b, half = core // 2, core % 2
        blk = res.results[core]["out"]  # (2048, 4096)
        out[b, half * ROWS * N:(half + 1) * ROWS * N, 0] = blk.ravel()
    if _trace:
        return out, res
    return out
